# revision 15
# baseline (speedup 1.0000x reference)
"""AdaptiveCompressionLayer kernel for 8 TRN2 NeuronCores.

Strategy (expert-routed data parallel, collapsed + centered experts):
  - Host: bucket tokens by importance score (>0.8 / >0.4 / rest), gather
    tokens into per-expert groups, split evenly across 8 cores with fixed
    per-expert capacities, and pre-transpose the routed activations to
    [H, T_pad] so the device needs no on-chip transposes.
  - Experts 0/1 have hc (691/537) > H/2, so the two-stage
    compress->decompress is MORE flops than the collapsed single matmul:
    host-precompute W_e = Wc_e @ Wd_e  [H, H]  and  b_e = bc_e@Wd_e + bd_e.
  - LayerNorm mean-centering is folded into the weights on the host:
    W' = W (I - J/H) centers every output row exactly, so the device
    LayerNorm needs no mean subtraction — just  out = y * rsqrt(var+eps).
    (Expert 2's decompressor image and all biases are centered the same
    way.)
  - Device heavy path (e0/e1): one dense [128tok x 768] = x^T.T @ W' per
    subtile (6 K-chunks, N split 512+256 for PSUM banks) — full PE
    utilization, no ragged tails.  The centered bias b' is folded in
    with a K=1 ones-row matmul (rhs = b' broadcast row); two subtiles'
    bias rows run concurrently as row-tiles at partition offsets 0/32.
  - Device light path (e2, hc=76 < H/2) keeps the factored two stages:
        Z^T = Wc2^T @ X^T  (one M=76 chunk), += bc2 via ACT copy,
        Y   = Z^T.T @ [Wd2; bd2]'  (ones-row trick folds the bias)
  - LayerNorm per 128-token subtile: bn_stats/bn_aggr (DVE), sqrt (ACT),
    one shared reciprocal per subtile-pair (DVE), scale-only normalize
    psum->sbuf bf16 (ACT).
  - The program is emitted as a WEAVE of subtile-pair tasks: heavy pairs
    alternate with e2 mm1 / e2 mm2-pair tasks so PE work, PSUM demand
    and LayerNorm (DVE/ACT) load stay uniform — no per-group bursts.
    The weave starts with e2-only tasks (tiny weights) while the big
    collapsed W images stream in, and ends heavy so DVE drains under PE.
  - Weight/const DMAs all issue up front, split across the scalar and
    gpsimd queues (xt loads own the sync queue; stores go on gpsimd).
  - Host: scatter valid rows back to the original token order.

No cross-core communication: routing is per-token, weights replicated.
"""
import sys

sys.path.insert(0, "/opt/trn_rl_repo")

import numpy as np
import ml_dtypes

BF16 = ml_dtypes.bfloat16

H = 768
HC2 = 76  # expert-2 bottleneck (only expert kept factored)
S = 65536
EPS = 1e-5
N_CORES = 8
GROUP = 512
CAPS = (1792, 3328, 3328)  # default; kernel() tightens from actual counts

TRACE = False
LAST_RESULT = None
USE_POW = False

_NC_CACHE = {}


def _groups(caps):
    per_e = []
    offs = (0, caps[0], caps[0] + caps[1])
    for e in range(3):
        glist = []
        t = 0
        while t < caps[e]:
            gsz = min(GROUP, caps[e] - t)
            glist.append((e, offs[e] + t, gsz))
            t += gsz
        per_e.append(glist)
    return per_e


def _weave(caps):
    """Task list: ('h', group, s) heavy subtile, ('mm1', group, None) e2
    compress stage, ('l', group, s) e2 decompress subtile.  Single
    subtile per task so each task holds exactly one PSUM tile — the
    3-deep PSUM pool then gives ~2 tasks of runway before a slot is
    reused, which exceeds the LayerNorm pipeline latency.  e2-only
    prologue while the big collapsed weights stream in; lights spread
    (never two adjacent); heavy-only tail."""
    per_e = _groups(caps)
    e0g, e1g, e2g = per_e
    # smallest e2 group first: its xt lands soonest, PE starts earlier
    e2g = sorted(e2g, key=lambda g: g[2])
    h1, h0 = list(e1g), list(e0g)
    tail_small = [g for g in h0 + h1 if g[2] < 512][-1:]
    h0 = [g for g in h0 if g not in tail_small]
    h1 = [g for g in h1 if g not in tail_small]
    heavy = []
    ratio = max(1, len(h1) // max(1, len(h0)))
    while h1 or h0:
        for _ in range(ratio):
            if h1:
                heavy.append(h1.pop(0))
        if h0:
            heavy.append(h0.pop(0))
    heavy.extend(tail_small)
    htasks = []
    for g in heavy:
        for s in range(g[2] // 128):
            htasks.append(("h", g, s))
    # light sequence with one-group mm1 lookahead
    lseq = []
    for gi, g in enumerate(e2g):
        nsub = g[2] // 128
        if gi == 0:
            lseq.append(("mm1", g, None))
        if nsub:
            lseq.append(("l", g, 0))
        if gi + 1 < len(e2g):
            lseq.append(("mm1", e2g[gi + 1], None))
        for s in range(1, nsub):
            lseq.append(("l", g, s))
    if not htasks:
        return lseq
    nstart = min(7, len(lseq))
    weave = list(lseq[:nstart])
    li = nstart
    reserve = min(6, len(htasks) - 1)
    rate = (len(lseq) - li) / max(1, len(htasks) - reserve)
    acc = 0.0
    for hi, ht in enumerate(htasks):
        weave.append(ht)
        if hi < len(htasks) - reserve:
            acc += rate
            if acc >= 1.0 and li < len(lseq):
                weave.append(lseq[li])
                li += 1
                acc -= 1.0
    # leftovers: spread before the last reserve heavies, one per heavy
    while li < len(lseq):
        weave.insert(len(weave) - reserve, lseq[li])
        li += 1
        reserve = max(1, reserve - 1)
    return weave


def _build(apply_gb: bool, caps=CAPS):
    import concourse.mybir as mybir
    import concourse.tile as tile
    from concourse import bacc

    f32 = mybir.dt.float32
    bf16 = mybir.dt.bfloat16
    AF = mybir.ActivationFunctionType
    ALU = mybir.AluOpType

    tpad = sum(caps)

    nc = bacc.Bacc(None, target_bir_lowering=False)

    xt_d = nc.declare_dram_parameter("xt", [H, tpad], bf16, isOutput=False)
    # collapsed+centered expert weights [128, 6*H]: [p, c*H+h] = W'_e[c*128+p, h]
    w_d = [
        nc.declare_dram_parameter(f"w{e}", [128, 6 * H], bf16, isOutput=False)
        for e in range(2)
    ]
    wc2_d = nc.declare_dram_parameter("wc2", [128, 6 * HC2], bf16, isOutput=False)
    wd2_d = nc.declare_dram_parameter("wd2", [128, H], bf16, isOutput=False)
    bc2_d = nc.declare_dram_parameter("bc2", [128, 1], f32, isOutput=False)
    bb_d = nc.declare_dram_parameter("bb", [2, H], bf16, isOutput=False)
    if apply_gb:
        gb_d = nc.declare_dram_parameter("gb", [2, H], f32, isOutput=False)
    out_d = nc.declare_dram_parameter("out", [tpad, H], bf16, isOutput=True)

    with tile.TileContext(nc) as tc:
        from contextlib import ExitStack

        with ExitStack() as ctx:
            wpool = ctx.enter_context(tc.tile_pool(name="weights", bufs=1))
            cpool = ctx.enter_context(tc.tile_pool(name="consts", bufs=1))
            xpool = ctx.enter_context(tc.tile_pool(name="xt", bufs=8))
            zpsum = ctx.enter_context(tc.tile_pool(name="zpsum", bufs=2, space="PSUM"))
            zpool = ctx.enter_context(tc.tile_pool(name="zsb", bufs=4))
            ypsum = ctx.enter_context(tc.tile_pool(name="ypsum", bufs=3, space="PSUM"))
            ypsum2 = ctx.enter_context(tc.tile_pool(name="ypsum2", bufs=3, space="PSUM"))
            opool = ctx.enter_context(tc.tile_pool(name="osb", bufs=6))
            lnpool = ctx.enter_context(tc.tile_pool(name="ln", bufs=8))

            # ---- weight tiles ----
            w_sb = [None] * 2
            for e in range(2):
                w_sb[e] = wpool.tile([128, 6, H], bf16, tag=f"w{e}", name=f"w_sb{e}")
            wc2_sb = wpool.tile([128, 6, HC2], bf16, tag="wc2", name="wc2_sb")
            wd2_sb = wpool.tile([128, H], bf16, tag="wd2", name="wd2_sb")
            bc2_sb = cpool.tile([128, 1], f32)
            bb_sb = cpool.tile([128, 2, H], bf16)

            # small consts on the scalar queue; wc2 + the big collapsed W
            # images ride the sync queue interleaved with the first few e2
            # xt loads (the only queue with full fan-out bandwidth), so w1
            # lands right as the e2 prologue drains.
            nc.scalar.dma_start(out=bc2_sb, in_=bc2_d[:, :])
            nc.scalar.dma_start(out=wd2_sb, in_=wd2_d[:, :])
            nc.scalar.dma_start(out=bb_sb, in_=bb_d.ap().partition_broadcast(128))
            nc.sync.dma_start(
                out=wc2_sb, in_=wc2_d.ap().rearrange("p (c h) -> p c h", c=6)
            )
            if apply_gb:
                gb_sb = cpool.tile([128, 2, H], f32)
                nc.gpsimd.dma_start(
                    out=gb_sb, in_=gb_d.ap().partition_broadcast(128)
                )

            eps_t = cpool.tile([128, 1], f32)
            nc.vector.memset(eps_t, EPS)
            ones_t = cpool.tile([128, 128], bf16)
            nc.vector.memset(ones_t, 1.0)

            # PE warm-up: dummy matmuls during the initial weight DMA wait
            # keep the HAM activity window hot so real matmuls start at
            # full clock.
            warm = cpool.tile([128, 512], bf16, name="warm")
            nc.vector.memset(warm, 0.0)
            warm_ps = zpsum.tile([128, 512], f32, tag="pz", name="warm_ps")
            for _w in range(20):
                nc.tensor.matmul(
                    warm_ps,
                    lhsT=warm[:, 0:128],
                    rhs=warm,
                    start=(_w == 0),
                    stop=(_w == 19),
                )
            xt_r = xt_d.ap().rearrange("(c p) t -> p c t", p=128)

            def ln_tail(stats, py5, py2, o_t):
                """stats [128,2,6] already computed per region; finish:
                var -> sqrt (ACT) -> reciprocal (DVE) -> scale-only
                normalize psum->sbuf bf16 on ACT, one op per region tile."""
                mv = lnpool.tile([128, 2], f32, tag="mv")
                nc.vector.bn_aggr(out=mv, in_=stats)
                rstd = lnpool.tile([128, 1], f32, tag="rstd")
                nc.scalar.activation(
                    out=rstd, in_=mv[:, 1:2], func=AF.Sqrt, bias=eps_t, scale=1.0
                )
                nc.vector.reciprocal(out=rstd, in_=rstd)
                nc.scalar.activation(
                    out=o_t[:, 0:384],
                    in_=py5,
                    func=AF.Identity,
                    bias=0.0,
                    scale=rstd[:, 0:1],
                )
                nc.scalar.activation(
                    out=o_t[:, 384:768],
                    in_=py2,
                    func=AF.Identity,
                    bias=0.0,
                    scale=rstd[:, 0:1],
                )
                if apply_gb:
                    nc.gpsimd.tensor_tensor(
                        out=o_t, in0=o_t, in1=gb_sb[:, 0, :], op=ALU.mult
                    )
                    nc.vector.tensor_add(o_t, o_t, gb_sb[:, 1, :])

            # ---- per-group device state, created lazily at first touch ----
            gstate = {}
            weave = _weave(caps)
            guniq = []
            for _, g, _s in weave:
                if g not in guniq:
                    guniq.append(g)
            last2 = set(guniq[-2:])

            def touch(g):
                if g in gstate:
                    return gstate[g]
                e, tok0, gsz = g
                xt_t = xpool.tile([128, 6, gsz], bf16, tag="xt", name=f"xt_{tok0}")
                nc.sync.dma_start(out=xt_t, in_=xt_r[:, :, tok0 : tok0 + gsz])
                o_t = opool.tile(
                    [128, gsz // 128, H], bf16, tag="o", name=f"o_{tok0}"
                )
                st = {
                    "xt": xt_t,
                    "o": o_t,
                    "done": 0,
                    "zt": None,
                }
                gstate[g] = st
                return st

            # prefetch the first groups' xt loads with the big collapsed W
            # images interleaved on the same (sync) queue: w1 lands as the
            # e2 prologue drains, w0 before the first e0 subtile.
            for gg in guniq[0:2]:
                touch(gg)
            nc.sync.dma_start(
                out=w_sb[1], in_=w_d[1].ap().rearrange("p (c h) -> p c h", c=6)
            )
            for gg in guniq[2:4]:
                touch(gg)
            nc.sync.dma_start(
                out=w_sb[0], in_=w_d[0].ap().rearrange("p (c h) -> p c h", c=6)
            )

            def finish(g, st, s):
                e, tok0, gsz = g
                st["done"] += 1
                if g in last2:
                    # tail groups store per subtile so the final store
                    # starts as soon as its normalize lands
                    nc.gpsimd.dma_start(
                        out=out_d[tok0 + s * 128 : tok0 + (s + 1) * 128, :],
                        in_=st["o"][:, s, :],
                    )
                elif st["done"] == gsz // 128:
                    nc.gpsimd.dma_start(
                        out=out_d[tok0 : tok0 + gsz, :].rearrange(
                            "(s p) h -> p s h", p=128
                        ),
                        in_=st["o"],
                    )

            for kind, g, subs in weave:
                e, tok0, gsz = g
                st = touch(g)
                if kind == "mm1":
                    zt = zpool.tile([128, 1, gsz], bf16, tag="zt")
                    st["zt"] = zt
                    # ones rows for the bias term: memset the 32-aligned
                    # window covering partition 76; the z copy below
                    # overwrites rows 0..75 inside it.
                    nc.gpsimd.memset(zt[64:96, 0, :], 1.0)
                    pz = zpsum.tile([128, gsz], f32, tag="pz")
                    for c in range(6):
                        nc.tensor.matmul(
                            pz[0:HC2, :],
                            lhsT=wc2_sb[:, c, :],
                            rhs=st["xt"][:, c, :],
                            start=(c == 0),
                            stop=(c == 5),
                        )
                    nc.scalar.activation(
                        out=zt[0:HC2, 0, :],
                        in_=pz[0:HC2, :],
                        func=AF.Identity,
                        bias=bc2_sb[0:HC2, 0:1],
                        scale=1.0,
                    )
                elif kind == "l":
                    s = subs
                    zt = st["zt"]
                    py5 = ypsum.tile([128, 384], f32, tag="py5")
                    py2 = ypsum2.tile([128, 384], f32, tag="py2")
                    stats = lnpool.tile([128, 2, 6], f32, tag="stats")
                    # region-major: each region matmul completes into its
                    # own PSUM tile, then its bn_stats overlaps the next
                    # region's matmul (no tile-level conflict)
                    for ri, (py, n0, nn) in enumerate(
                        ((py5, 0, 384), (py2, 384, 384))
                    ):
                        nc.tensor.matmul(
                            py,
                            lhsT=zt[0 : HC2 + 1, 0, s * 128 : (s + 1) * 128],
                            rhs=wd2_sb[0 : HC2 + 1, n0 : n0 + nn],
                            start=True,
                            stop=True,
                        )
                        nc.vector.bn_stats(out=stats[:, ri, :], in_=py)
                    ln_tail(stats, py5, py2, st["o"][:, s, :])
                    finish(g, st, s)
                else:  # heavy subtile
                    s = subs
                    py5 = ypsum.tile([128, 384], f32, tag="py5")
                    py2 = ypsum2.tile([128, 384], f32, tag="py2")
                    stats = lnpool.tile([128, 2, 6], f32, tag="stats")
                    for ri, (py, n0, nn) in enumerate(
                        ((py5, 0, 384), (py2, 384, 384))
                    ):
                        for c in range(6):
                            nc.tensor.matmul(
                                py,
                                lhsT=st["xt"][:, c, s * 128 : (s + 1) * 128],
                                rhs=w_sb[e][:, c, n0 : n0 + nn],
                                start=(c == 0),
                                stop=False,
                            )
                        nc.tensor.matmul(
                            py,
                            lhsT=ones_t[0:1, 0:128],
                            rhs=bb_sb[0:1, e, n0 : n0 + nn],
                            start=False,
                            stop=True,
                            tile_position=(0, 0),
                        )
                        nc.vector.bn_stats(out=stats[:, ri, :], in_=py)
                    ln_tail(stats, py5, py2, st["o"][:, s, :])
                    finish(g, st, s)
    nc.finalize()
    return nc


def _get_nc(apply_gb: bool, caps):
    key = (apply_gb, caps)
    if key not in _NC_CACHE:
        _NC_CACHE[key] = _build(apply_gb, caps=caps)
    return _NC_CACHE[key]


def _pack_weights(inputs):
    base = {}
    # collapsed experts 0/1: W = Wc@Wd, b = bc@Wd + bd (f64 precompute),
    # then fold LayerNorm mean-centering: W' = W - rowmean(W), b' = b - mean(b)
    bb = np.zeros((2, H), np.float64)
    for e in range(2):
        wc = np.asarray(inputs[f"Wc{e}"], dtype=np.float64)
        bc = np.asarray(inputs[f"bc{e}"], dtype=np.float64)
        wd = np.asarray(inputs[f"Wd{e}"], dtype=np.float64)
        bd = np.asarray(inputs[f"bd{e}"], dtype=np.float64)
        W = wc @ wd  # [H, H]
        b = bc @ wd + bd  # [H]
        W = W - W.mean(axis=1, keepdims=True)
        b = b - b.mean()
        wi = W.reshape(6, 128, H).transpose(1, 0, 2)  # [p, c, h]
        base[f"w{e}"] = np.ascontiguousarray(wi.reshape(128, 6 * H)).astype(BF16)
        bb[e] = b
    base["bb"] = np.ascontiguousarray(bb).astype(BF16)
    # expert 2 factored; center the decompressor image rows (incl. bias row)
    wc2 = np.asarray(inputs["Wc2"], dtype=np.float32)  # [H, 76]
    bc2 = np.asarray(inputs["bc2"], dtype=np.float32)
    wd2 = np.asarray(inputs["Wd2"], dtype=np.float64)  # [76, H]
    bd2 = np.asarray(inputs["bd2"], dtype=np.float64)
    wc2i = wc2.reshape(6, 128, HC2).transpose(1, 0, 2)
    base["wc2"] = np.ascontiguousarray(wc2i.reshape(128, 6 * HC2)).astype(BF16)
    wd2i = np.zeros((128, H), np.float64)
    wd2i[0:HC2] = wd2
    wd2i[HC2] = bd2
    wd2i[0 : HC2 + 1] -= wd2i[0 : HC2 + 1].mean(axis=1, keepdims=True)
    base["wd2"] = np.ascontiguousarray(wd2i).astype(BF16)
    bc2p = np.zeros((128, 1), np.float32)
    bc2p[0:HC2, 0] = bc2
    base["bc2"] = bc2p
    return base


def kernel(**inputs):
    global LAST_RESULT
    from concourse.bass_utils import run_bass_kernel_spmd

    hs = np.ascontiguousarray(np.asarray(inputs["hidden_states"], dtype=np.float32))
    sc = np.asarray(inputs["importance_scores"], dtype=np.float32)
    gamma = np.asarray(inputs["gamma"], dtype=np.float32)
    beta = np.asarray(inputs["beta"], dtype=np.float32)

    # routing (must match f32 comparison semantics of the reference)
    m0 = sc > np.float32(0.8)
    m1 = (sc > np.float32(0.4)) & ~m0
    bucket = np.where(m0, 0, np.where(m1, 1, 2)).astype(np.int64)
    idx = [np.flatnonzero(bucket == e) for e in range(3)]
    splits = [np.array_split(idx[e], N_CORES) for e in range(3)]

    # tight per-core caps: max per-core count rounded up to 128
    caps = tuple(
        int(-(-max(len(p) for p in splits[e]) // 128) * 128) for e in range(3)
    )
    tpad = sum(caps)
    offs = (0, caps[0], caps[0] + caps[1])

    gidx = np.zeros((N_CORES, tpad), np.int64)
    valid = np.zeros((N_CORES, tpad), bool)
    for c in range(N_CORES):
        for e in range(3):
            p = splits[e][c]
            o = offs[e]
            gidx[c, o : o + len(p)] = p
            valid[c, o : o + len(p)] = True

    apply_gb = not (np.all(gamma == 1.0) and np.all(beta == 0.0))
    nc = _get_nc(apply_gb, caps)

    base = _pack_weights(inputs)
    if apply_gb:
        base["gb"] = np.ascontiguousarray(np.stack([gamma, beta], axis=0))

    in_maps = []
    for c in range(N_CORES):
        xc = hs[gidx[c]]  # [TPAD, H]
        m = dict(base)
        m["xt"] = np.ascontiguousarray(xc.T.astype(BF16))
        in_maps.append(m)

    # The device occasionally returns corrupted (non-finite) results or
    # raises an unrecoverable-state error; inputs are finite and LayerNorm
    # output is always finite, so retry in both cases.
    for attempt in range(4):
        try:
            res = run_bass_kernel_spmd(
                nc, in_maps, core_ids=list(range(N_CORES)), trace=TRACE
            )
        except Exception:
            if attempt == 3:
                raise
            import time as _time

            _time.sleep(2.0)
            continue
        LAST_RESULT = res
        out = np.empty((S, H), np.float32)
        for c in range(N_CORES):
            v = valid[c]
            out[gidx[c][v]] = res.results[c]["out"][v]
        if np.isfinite(out).all():
            break
    return out
